# revision 2
# baseline (speedup 1.0000x reference)
"""MoE (top-2 of 8 experts, swiglu MLP) Trainium2 kernel — v3.

2 H-halves x 4 token-quarters sharding (see v2 docstring), plus:
  - all segment/chunk sizes rounded to multiples of 16 so every matmul's
    moving operand is 16B-aligned (odd widths measured ~7% slower);
  - w13 laid out [E, HT2, P, DSUB, 2P] in DRAM so its DMA moves 4KB
    contiguous runs per partition instead of 512B packets;
  - DMA queues split by role so no compute instruction can queue behind a
    blocking DMA enqueue (strict-FIFO engine queues): sync ring carries the
    w13 stream, gpsimd ring carries x/x3/w2, vector carries the small y
    write-back, scalar runs only the SILU activations;
  - segments processed smallest-first so the final (largest, 2-chunk)
    segment leaves the shortest un-overlapped stage-2 tail;
  - first segment begins with a 256-column chunk (and column-split x DMA)
    so the first matmul group starts as early as possible.
"""

import numpy as np

B, S, D, HID, E, TOPK = 4, 2048, 1024, 2816, 8, 2
P = 128
DSUB = D // P            # 8 d-subtiles (contraction)
NDT = D // P             # 8 d-tiles (stage-2 output partitions)
HHALF = HID // 2         # 1408
HT2 = HHALF // P         # 11 h-tiles per half
NCORES = 8
NQ = 4                   # token quarters
NH = 2                   # H halves

_nc_cache: dict[tuple, object] = {}


def _chunks_of(seg, first=False):
    """Moving-dim chunks: multiples of 8 (16B), <=512, balanced halves."""
    assert seg % 8 == 0
    if first and seg > 256:
        return [(0, 256)] + [(lo + 256, w) for lo, w in _chunks_of(seg - 256)]
    if seg <= 512:
        return [(0, seg)]
    m = seg // 8
    h1 = 8 * ((m + 1) // 2)
    return [(0, h1), (h1, seg - h1)]


def _build(segs):
    import concourse.tile as tile
    from concourse import bacc, mybir

    F32, F16 = mybir.dt.float32, mybir.dt.float16
    SILU = mybir.ActivationFunctionType.Silu
    MULT = mybir.AluOpType.mult

    T = sum(segs)
    segmax = max(segs)

    nc = bacc.Bacc("TRN2", target_bir_lowering=False, debug=False,
                   num_devices=NCORES)
    xT = nc.dram_tensor("xT", [D, T], F16, kind="ExternalInput")
    x3T = nc.dram_tensor("x3T", [D, T], F16, kind="ExternalInput")
    w13 = nc.dram_tensor("w13", [E, HT2, P, DSUB, 2 * P], F16,
                         kind="ExternalInput")
    w2 = nc.dram_tensor("w2", [E, HT2, P, D], F16, kind="ExternalInput")
    y = nc.dram_tensor("y", [D, T], F16, kind="ExternalOutput")

    xT_r = xT.ap().rearrange("(do dp) t -> dp do t", dp=P)
    x3T_r = x3T.ap().rearrange("(do dp) t -> dp do t", dp=P)
    w13_r = w13.ap().rearrange("e ht p ds z -> p e ht ds z")
    w2_r = w2.ap().rearrange("e ht p d -> p e ht d")
    y_r = y.ap().rearrange("(dt dp) t -> dp dt t", dp=P)

    offs = np.cumsum([0] + list(segs))[:-1]

    with tile.TileContext(nc) as tc:
        with tc.tile_pool(name="xpool", bufs=2) as xpool, \
             tc.tile_pool(name="wpool", bufs=10) as wpool, \
             tc.tile_pool(name="w2pool", bufs=2) as w2pool, \
             tc.tile_pool(name="gpool", bufs=2) as gpool, \
             tc.tile_pool(name="ypool", bufs=2) as ypool, \
             tc.tile_pool(name="tpool", bufs=2) as tpool, \
             tc.tile_pool(name="psum", bufs=1, space="PSUM") as psum, \
             tc.tile_pool(name="psum2", bufs=2, space="PSUM") as psum2, \
             tc.tile_pool(name="wupool", bufs=1) as wupool, \
             tc.tile_pool(name="psumw", bufs=1, space="PSUM") as psumw:
            # PE warmup: ~36 dummy matmuls from a memset tile keep the PE
            # busy from engine boot until real data lands, tripping the HAM
            # clock un-throttle (4/8 -> 8/8) before the first real matmul.
            wsrc = wupool.tile([P, 64], F16, tag="wsrc")
            nc.vector.memset(wsrc[:], 0.25)
            pw = psumw.tile([64, 64], F32, tag="pwarm")
            for _ in range(36):
                nc.tensor.matmul(pw[:, :], wsrc[:, :64], wsrc[:, :64],
                                 start=True, stop=True)
            for si in range(E):
                e, seg, off = si, int(segs[si]), int(offs[si])
                if seg == 0:
                    continue
                first = si == 0
                chunks = _chunks_of(seg, first=first)

                # token activations (both copies); per-ds DMAs so the first
                # matmul group only waits on ds=0 (col-split for segment 0)
                xc = xpool.tile([P, DSUB, segmax], F16, tag="x1")
                x3c = xpool.tile([P, DSUB, segmax], F16, tag="x3")
                if first:
                    for ds in range(DSUB):
                        nc.gpsimd.dma_start(xc[:, ds, :256],
                                            xT_r[:, ds, off:off + 256])
                    for ds in range(DSUB):
                        nc.gpsimd.dma_start(xc[:, ds, 256:seg],
                                            xT_r[:, ds, off + 256:off + seg])
                else:
                    for ds in range(DSUB):
                        nc.gpsimd.dma_start(xc[:, ds, :seg],
                                            xT_r[:, ds, off:off + seg])
                # segment 0's x3 goes on the (otherwise idle at boot) scalar
                # ring so the w3-path matmuls aren't starved behind xc
                dma_x3 = nc.scalar.dma_start if first else nc.gpsimd.dma_start
                for ds in range(DSUB):
                    dma_x3(x3c[:, ds, :seg], x3T_r[:, ds, off:off + seg])
                # stage-2 stationary operand for this segment
                w2sb = w2pool.tile([P, HT2, D], F16, tag="w2")
                nc.gpsimd.dma_start(w2sb[:], w2_r[:, e, :, :])

                g = gpool.tile([P, HT2, segmax], F16, tag="g")
                for ht in range(HT2):
                    wc = wpool.tile([P, DSUB, 2 * P], F16, tag="w13")
                    if first and ht == 0:
                        # cold start: per-ds strips so the very first matmul
                        # only waits on a 32KB transfer
                        for ds in range(DSUB):
                            nc.sync.dma_start(wc[:, ds, :P],
                                              w13_r[:, e, ht, ds, :P])
                        nc.sync.dma_start(wc[:, :, P:],
                                          w13_r[:, e, ht, :, P:])
                    else:
                        nc.sync.dma_start(wc[:], w13_r[:, e, ht, :, :])
                    phs = []
                    for half, xin in ((0, xc), (1, x3c)):
                        for ci, (lo, w) in enumerate(chunks):
                            ph = psum.tile([P, 512], F32,
                                           tag=f"ph{half}{ci}")
                            for ds in range(DSUB):
                                nc.tensor.matmul(
                                    ph[:, :w],
                                    wc[:, ds, half * P:half * P + P],
                                    xin[:, ds, lo:lo + w],
                                    start=(ds == 0), stop=(ds == DSUB - 1))
                            phs.append((ph, lo, w))
                    nch = len(chunks)
                    for (ph1, lo, w), (ph3, _, _) in zip(phs[:nch],
                                                         phs[nch:]):
                        tmp = tpool.tile([P, 512], F32, tag="tmp")
                        nc.scalar.activation(tmp[:, :w], ph1[:, :w], SILU)
                        nc.vector.tensor_tensor(g[:, ht, lo:lo + w],
                                                tmp[:, :w], ph3[:, :w],
                                                MULT)

                ysb = ypool.tile([P, NDT, segmax], F16, tag="y")
                last_seg = si == E - 1
                for ic, (lo, w) in enumerate(chunks):
                    drain = last_seg and ic == len(chunks) - 1
                    for dt in range(NDT):
                        py = psum2.tile([P, 512], F32, tag="py")
                        for ht in range(HT2):
                            nc.tensor.matmul(
                                py[:, :w],
                                w2sb[:, ht, dt * P:(dt + 1) * P],
                                g[:, ht, lo:lo + w],
                                start=(ht == 0), stop=(ht == HT2 - 1))
                        nc.vector.tensor_copy(ysb[:, dt, lo:lo + w],
                                              py[:, :w])
                        if drain:
                            # final chunk: write back per d-tile so the y DMA
                            # overlaps the remaining stage-2 groups
                            nc.scalar.dma_start(
                                y_r[:, dt, off + lo:off + lo + w],
                                ysb[:, dt, lo:lo + w])
                    if not drain:
                        nc.scalar.dma_start(
                            y_r[:, :, off + lo:off + lo + w],
                            ysb[:, :, lo:lo + w])
    nc.compile()
    return nc


def _get_nc(segs):
    key = tuple(segs)
    if key not in _nc_cache:
        _nc_cache[key] = _build(segs)
    return _nc_cache[key]


def _route(xt, gate_w):
    T = xt.shape[0]
    scores = xt.astype(np.float64) @ gate_w.astype(np.float64).T
    ar = np.arange(T)
    i1 = np.argmax(scores, 1)
    s1 = scores[ar, i1]
    scores[ar, i1] = -np.inf
    i2 = np.argmax(scores, 1)
    s2 = scores[ar, i2]
    e2 = np.exp(s2 - s1)
    denom = 1.0 + e2
    return i1, i2, 1.0 / denom, e2 / denom


def _ensure_axon_hooks():
    """bass_utils imports antenv.axon_hooks when tracing is requested
    (e.g. BASS_TRACE=1); some images lack that module. Register a shim
    backed by the boot ctypes NTFF hook so tracing works instead of
    crashing."""
    try:
        import antenv.axon_hooks  # noqa: F401
        return
    except ImportError:
        pass
    import sys
    import types
    hook = None
    try:
        from trn_agent_boot.trn_boot import _ntff_profile_via_ctypes
        hook = _ntff_profile_via_ctypes("/opt/axon/libaxon_pjrt.so")
    except Exception:
        hook = None
    try:
        import antenv
    except ImportError:
        return
    mod = types.ModuleType("antenv.axon_hooks")
    mod.get_axon_ntff_profile_hook = lambda: hook
    mod.set_axon_ntff_profile_hook = lambda h: None
    sys.modules["antenv.axon_hooks"] = mod
    antenv.axon_hooks = mod


def kernel(x, gate_w, w1, w3, w2):
    _ensure_axon_hooks()
    from concourse.bass_utils import run_bass_kernel_spmd

    x = np.asarray(x, dtype=np.float32)
    gate_w = np.asarray(gate_w, dtype=np.float32)
    w1 = np.asarray(w1, dtype=np.float32)
    w3 = np.asarray(w3, dtype=np.float32)
    w2 = np.asarray(w2, dtype=np.float32)

    b, s, d = x.shape
    T = b * s
    xt = x.reshape(T, d)
    i1, i2, wa, wb = _route(xt, gate_w)

    idxs, coefs = [], []
    for e in range(E):
        m1 = i1 == e
        m2 = i2 == e
        cf = np.where(m1, wa, 0.0) + np.where(m2, wb, 0.0)
        idx = np.nonzero(m1 | m2)[0]
        idxs.append(idx)
        coefs.append(cf[idx].astype(np.float32))

    # per-expert quarter capacity, multiple of 16; process smallest-first
    def _cap(n):
        s = -(-n // NQ)          # ceil(n / 4)
        return max(16, 8 * (-(-s // 8)))  # round up to multiple of 8 (16B)
    caps = [_cap(len(idxs[e])) for e in range(E)]
    order = sorted(range(E), key=lambda e: (caps[e], e))
    segs = [caps[e] for e in order]
    offs = np.cumsum([0] + segs)[:-1]
    Ttot = sum(segs)
    nc = _get_nc(segs)

    xtT = np.ascontiguousarray(xt.T.astype(np.float16))

    # per-quarter activations (shared by both H-half cores)
    qx, qx3, qmeta = [], [], []
    for q in range(NQ):
        xTq = np.zeros((D, Ttot), np.float16)
        x3Tq = np.zeros((D, Ttot), np.float16)
        meta = []
        for si, e in enumerate(order):
            se = segs[si]
            idx = idxs[e][q * se:(q + 1) * se]
            n = len(idx)
            cf = coefs[e][q * se:(q + 1) * se]
            o = int(offs[si])
            cols = xtT[:, idx]
            xTq[:, o:o + n] = cols
            x3Tq[:, o:o + n] = (cols.astype(np.float32)
                                * cf[None, :]).astype(np.float16)
            meta.append((idx, o, n))
        qx.append(xTq)
        qx3.append(x3Tq)
        qmeta.append(meta)

    # per-half weights
    hw13, hw2 = [], []
    for h in range(NH):
        lo = h * HHALF
        w13h = np.empty((E, D, HT2, 2 * P), np.float16)
        w13h[:, :, :, :P] = w1[:, :, lo:lo + HHALF].reshape(E, D, HT2, P)
        w13h[:, :, :, P:] = w3[:, :, lo:lo + HHALF].reshape(E, D, HT2, P)
        # -> [E, HT2, P(dp), DSUB(do), 2P] for 4KB-contiguous DMA runs,
        # permuted into segment-processing order (device uses e = seg index)
        w13q = np.ascontiguousarray(
            w13h.reshape(E, DSUB, P, HT2, 2 * P).transpose(0, 3, 2, 1, 4)
            [order])
        w2h = np.ascontiguousarray(
            w2[:, lo:lo + HHALF, :].reshape(E, HT2, P, D).astype(np.float16)
            [order])
        hw13.append(w13q)
        hw2.append(w2h)

    in_maps = []
    for core in range(NCORES):
        h, q = core // NQ, core % NQ
        in_maps.append({
            "xT": qx[q],
            "x3T": qx3[q],
            "w13": hw13[h],
            "w2": hw2[h],
        })

    res = run_bass_kernel_spmd(nc, in_maps, core_ids=list(range(NCORES)))

    out = np.zeros((T, D), np.float32)
    for q in range(NQ):
        yq = (res.results[q]["y"].astype(np.float32)
              + res.results[NQ + q]["y"].astype(np.float32))
        for idx, o, n in qmeta[q]:
            if n:
                out[idx] += yq[:, o:o + n].T
    return out.reshape(b, s, d)
